# revision 7
# baseline (speedup 1.0000x reference)
"""MoE block (top-1 routing, shared FFN + per-expert LoRA) on 8 TRN2 NeuronCores.

v3: data-parallel over tokens (1024/core), weights replicated.

    logits = x @ gate_W.T + gate_b ; e* = argmax(logits)        (fp32-split)
    u      = x @ A_cat.T                 [N, 32]
    u_m    = u * onehot-mask(e*)
    inter  = relu(x @ wi_W.T + u_m @ B_cat + wi_b)              (bf16 matmul)
    out    = inter @ wo_W.T + wo_b                              (bf16 matmul)

vs v2 (268.4us -> ~261us at full clock):
- Phase 1 closes its PSUM banks at its own end; extraction runs under f0.
- The dx residual ships as fp8e4m3 (dx*2^12, 1MB instead of 2MB bf16) with
  the gate stationary also fp8 (g*2^6); its correction matmuls interleave
  into f1's k-loop and merge into the logits with one DVE op (2^-18 descale
  folded in), so the mask chain starts ~10us earlier than v2.
- Mask chain: one [40,128] transpose per token chunk covers both halves'
  logits, u is transposed beside them, and a single fused DVE op per chunk
  computes (logit==rowmax)*u; the transposed masked-u stages to SBUF on
  scalar (th0) and vector (th1) concurrently.
- All host tensors are pre-tiled into straight 2D-copy layouts (no gather
  elements); DMA issue order tuned so the two HW queues never block compute
  engines; wi chunks ride a 3-buffer pool so their transfers self-throttle.
- 26 boot warms span the ~9.5us 8-core HBM-contended front-DMA window, so
  HAM unthrottles once at ~12us and never re-throttles (the previous ~3.5us
  data-wait gap sat exactly at the HAM MID threshold and cost a half-clock
  window every run).
- All outputs on hardware DMA queues; final drain split across both queues;
  wo triple-buffered.
"""

import numpy as np
import ml_dtypes
from contextlib import ExitStack

import concourse.bass as bass
import concourse.tile as tile
from concourse import bacc, mybir
from concourse.bass_utils import run_bass_kernel_spmd
from concourse.masks import make_identity

F32 = mybir.dt.float32
BF16 = mybir.dt.bfloat16
F8E4 = mybir.dt.float8e4
BF = ml_dtypes.bfloat16
F8 = ml_dtypes.float8_e4m3
DX_SCALE = 4096.0        # dx shipped as e4m3(dx * 2^12)
G8_SCALE = 64.0          # gate stationary as e4m3(g * 2^6)
CORR_DESCALE = 1.0 / (DX_SCALE * G8_SCALE)

B, S, D, F, E, R = 4, 2048, 1024, 4096, 8, 4
NCORES = 8
NT = B * S          # 8192 tokens total
N = NT // NCORES    # 1024 tokens per core
ER = E * R          # 32 lora rows
KD = D // 128       # 8 contraction tiles over D
KF = F // 128       # 32 contraction tiles over F
TH = 2              # token halves (512 each)
P = 128

Relu = mybir.ActivationFunctionType.Relu
Identity = mybir.ActivationFunctionType.Identity
Add = mybir.AluOpType.add
Max = mybir.AluOpType.max
Mult = mybir.AluOpType.mult
IsEq = mybir.AluOpType.is_equal

TS = [slice(th * 512, (th + 1) * 512) for th in range(TH)]


def _emit(ctx: ExitStack, tc: tile.TileContext, io: dict):
    nc = tc.nc

    consts = ctx.enter_context(tc.tile_pool(name="consts", bufs=1))
    xpool = ctx.enter_context(tc.tile_pool(name="xpool", bufs=1))
    wipool = ctx.enter_context(tc.tile_pool(name="wipool", bufs=1))
    ipool = ctx.enter_context(tc.tile_pool(name="ipool", bufs=1))
    rwork = ctx.enter_context(tc.tile_pool(name="rwork", bufs=1))
    wop = ctx.enter_context(tc.tile_pool(name="wop", bufs=3))
    outp = ctx.enter_context(tc.tile_pool(name="outp", bufs=2))
    # 2 banks: router accumulators -> dx-corr th1 -> logit/mask transposes
    ppc = ctx.enter_context(tc.tile_pool(name="ppc", bufs=2, space="PSUM"))
    # 6 banks rotating for mm1/mm2 accumulation (3 f-singles in flight);
    # slot 1 (warm_ps) doubles as the dx-corr th0 scratch
    pp = ctx.enter_context(tc.tile_pool(name="pp", bufs=6, space="PSUM"))

    # ---------- warm-up (memset on idle gpsimd; PE spins from boot) ----------
    warm_src = consts.tile([P, 512], BF16, tag="warm")
    nc.gpsimd.memset(warm_src, 1.0)
    warm_ps = pp.tile([P, 512], F32, tag="pb", name="warm_ps")

    def warm(n):
        for _ in range(n):
            nc.tensor.matmul(warm_ps, lhsT=warm_src[:, 0:P], rhs=warm_src,
                             start=True, stop=True)

    identity = consts.tile([P, P], F32, tag="identity")
    make_identity(nc, identity)
    identity16 = consts.tile([P, P], BF16, tag="identity16")
    nc.vector.tensor_copy(identity16, identity)

    # ---------- consts (DMAs issued below in priority order) ----------
    # biases [128, 64] f32: cols 0:32 wi_b by f-tile, 32:40 wo_b by d-tile,
    # 40:41 gate_b by expert partition (rows 0:8), rest zero.
    biases_sb = consts.tile([P, 64], F32, tag="biases")
    wib_sb = biases_sb[:, 0:KF]
    wob_sb = biases_sb[:, KF:KF + KD]
    gb_col = biases_sb[0:E, 40:41]
    # cg [D, 72] bf16: cols 0:32 a16 (er-major), 32:40 g16, 40:64 zero,
    # 64:72 dg16  (router stationaries; dx-corr reuses the g16 columns)
    CW = 72
    cg_big = consts.tile([P, KD * CW], BF16, tag="cg")
    cg_sb = [cg_big[:, k * CW:(k + 1) * CW] for k in range(KD)]
    bT4_sb = consts.tile([P, F], BF16, tag="bT4")

    # ---------- input DMAs: S = sync queue, A = scalar queue ----------
    x16 = xpool.tile([P, KD * N], BF16, tag="x16")
    x16v = x16.rearrange("p (k t) -> p k t", k=KD)
    # dx8 split by token half, each in straight pre-tiled layout
    dx8t = [xpool.tile([P, KD * 512], F8E4, tag=f"dx8{th}", name=f"dx8{th}")
            for th in range(TH)]
    cg8 = consts.tile([P, KD * E], F8E4, tag="cg8")
    NC_WI = 8           # wi chunks: 4 f-tiles (512 f-cols) x all k each
    # 3 rotating buffers: chunk c+3's DMA transfer is gated on chunk c's
    # consumers, throttling the wi flood so it never starves the scalar
    # queue's dx8/bT4 transfers of HBM bandwidth (also saves 5MB SBUF)
    wi_c = [wipool.tile([P, KD * 512], BF16, tag="wic", bufs=3,
                        name=f"wic{c}")
            for c in range(NC_WI)]
    # dedicated f0-only chunk so f0 can start right at phase-1 end
    wi_f0 = wipool.tile([P, KD * P], BF16, tag="wif0")

    def wi_lhsT(k, f):
        if f == 0:
            return wi_f0[:, k * P:(k + 1) * P]
        c, fr = divmod(f, 4)
        return wi_c[c][:, k * 512 + fr * P:k * 512 + (fr + 1) * P]

    # DMA issue order is critical: issues cost ~0.7us each on the issuing
    # engine and >10 outstanding DMAs hit the semaphore pool.  Everything is
    # pre-tiled host-side into straight 2D-copy layouts (contiguous rows per
    # partition) so no front transfer pays gather-element slowdowns.  x and
    # cg go first on their queues; ALL wi chunks ride sync (no compute to
    # block there); scalar's issue list is short so extraction is never
    # stuck behind it.
    # S queue: x k01, x k23, wi_f0, c0, c1, c2, ..., c7
    # A queue: cg, x k45, x k67, cg8, dx8 th0, dx8 th1, biases, bT4
    x16_src4 = io["xT16"].rearrange("(q k p) t -> q p k t", q=4, p=P)
    nc.sync.dma_start(out=x16v[:, 0:2], in_=x16_src4[0])
    nc.scalar.dma_start(out=cg_big, in_=io["cgTt"])
    nc.scalar.dma_start(out=biases_sb, in_=io["biases"])
    nc.sync.dma_start(out=x16v[:, 2:4], in_=x16_src4[1])
    nc.scalar.dma_start(out=x16v[:, 4:6], in_=x16_src4[2])
    nc.scalar.dma_start(out=x16v[:, 6:8], in_=x16_src4[3])
    nc.sync.dma_start(out=wi_f0, in_=io["wif0t"])
    nc.scalar.dma_start(out=cg8, in_=io["cg8t"])
    nc.scalar.dma_start(out=dx8t[0], in_=io["dx8a"])
    nc.scalar.dma_start(out=dx8t[1], in_=io["dx8b"])
    nc.scalar.dma_start(out=bT4_sb, in_=io["bT4"])
    for c in range(NC_WI):
        nc.sync.dma_start(out=wi_c[c], in_=io["wiTt"][c])

    xk = [x16[:, k * N:(k + 1) * N] for k in range(KD)]
    cg8_sb = [cg8[:, k * E:(k + 1) * E] for k in range(KD)]

    # ---------- resident intermediates ----------
    inter_sb = [ipool.tile([P, N], BF16, tag=f"inter{f}", name=f"inter{f}")
                for f in range(KF)]
    # um_stack rows: 0:32 th0, 32:64 th1, 64:96 th0, 96:128 th1
    um_stack = rwork.tile([P, 512], BF16, tag="um")
    # router scratch (legal partition bases only: 0/32/64/96)
    ub2 = rwork.tile([64, 512], F32, tag="ub2")       # [0:32] th0, [32:64] th1
    cdlg = rwork.tile([40, 512], F32, tag="cdlg")     # [0:8] th0, [32:40] th1
    lgb1 = rwork.tile([40, 512], F32, tag="lgb1")     # x16-only logits (+gb)
    lgb = rwork.tile([40, 512], F32, tag="lgb")       # exact logits
    # rows 8:32 of lgb are never written but ARE streamed through the merged
    # [40,128] logit transpose -- zero them once so no poison flows
    nc.gpsimd.memset(lgb, 0.0)
    mrep = {th: rwork.tile([P, 4 * ER], BF16, tag=f"mr{th}", name=f"mr{th}")
            for th in range(TH)}

    # ---------- PE emission ----------
    # 26 boot warms: observed across runs, the 8 cores' simultaneous front
    # DMAs mean x k01 + cg are not ready until ~9.5us after the first PE
    # instruction; warms must span that or the resulting ~3.5us gap sits
    # right at the HAM MID threshold and costs a half-clock window
    warm(26)

    # phase 1: router+lora stationaries, k in DMA-arrival order.
    # pcu[th] rows = [u | logits16 | pad | dlogits(dg16)]
    pcu = [ppc.tile([CW, 512], F32, tag="pc", name=f"pcu{th}")
           for th in range(TH)]
    PH1_ORDER = [0, 1, 4, 5, 2, 3, 6, 7]
    for i, k in enumerate(PH1_ORDER):
        for th in range(TH):
            nc.tensor.matmul(pcu[th], lhsT=cg_sb[k], rhs=xk[k][:, TS[th]],
                             start=(i == 0), stop=(i == len(PH1_ORDER) - 1))
        if 1 <= i <= 6:
            # pad every x-arrival seam so HAM never sees a DMA-paced gap
            warm(1)

    # early extraction: scalar stages dlogits (+gate_b) and u out of PSUM;
    # DVE forms x16-logits.  Runs under f0 mains; pcu banks free after this.
    def extract_emit(th):
        dlg_t = cdlg[0:8, :] if th == 0 else cdlg[32:40, :]
        nc.scalar.activation(dlg_t, pcu[th][64:72, :], Identity, bias=gb_col)
        nc.scalar.activation(ub2[32 * th:32 * (th + 1), :],
                             pcu[th][0:32, :], Identity)
        lg1 = lgb1[0:8, :] if th == 0 else lgb1[32:40, :]
        nc.vector.tensor_add(lg1, pcu[th][32:40, :], dlg_t)

    extract_emit(0)
    extract_emit(1)

    def f_mains(f, ps, ks):
        for k in ks:
            for th in range(TH):
                nc.tensor.matmul(ps[th], lhsT=wi_lhsT(k, f),
                                 rhs=xk[k][:, TS[th]],
                                 start=(k == 0), stop=False)

    def quad_stops(f0, f1, ps4):
        # 4 concurrent row-tiled stop matmuls: (f0,th0)(f0,th1)(f1,th0)(f1,th1)
        for j, f in ((0, f0), (1, f0), (2, f1), (3, f1)):
            nc.tensor.matmul(ps4[j], lhsT=bT4_sb[32 * j:32 * (j + 1),
                                                 f * P:(f + 1) * P],
                             rhs=um_stack[32 * j:32 * (j + 1), :],
                             start=False, stop=True,
                             tile_position=(32 * j, 0))

    def f_acts(f, ps):
        # th0 on scalar, th1 on vector
        nc.scalar.activation(inter_sb[f][:, TS[0]], ps[0], Relu,
                             bias=wib_sb[:, f:f + 1])
        nc.vector.tensor_scalar(inter_sb[f][:, TS[1]], ps[1],
                                wib_sb[:, f:f + 1], 0.0, Add, Max)

    def f_tiles(f):
        return [pp.tile([P, 512], F32, tag="pb", name=f"p{f}_{th}")
                for th in range(TH)]

    # --- f0 mains (dedicated wi_f0 chunk) right at phase-1 end
    warm(1)
    ps_f = {0: f_tiles(0)}
    f_mains(0, ps_f[0], range(KD))

    # phase 2: dx-residual logit correction (fp8 dx @ fp8 g stationary).
    # th0 into the warm bank (slot 1 of pp), th1 into pcu0's freed bank.
    # The matmuls interleave into f1's k-loop below.
    corr0 = warm_ps[0:8, :]
    corr1 = ppc.tile([8, 512], F32, tag="pc", name="corr1")

    # --- mask path emitters ---------------------------------------------
    # One PSUM bank holds ALL the token-major router data: cols 0:160 are
    # the transposed logits (4 chunks x [40]: rows 0:8 th0, 32:40 th1),
    # cols 160:416 the transposed u (4 chunks x [64]: 0:32 th0, 32:64 th1).
    # Per (th, chunk) ONE fused DVE op computes (logit==rowmax) * u.
    trps_t = {}

    def trps_bank():
        trps_t[0] = ppc.tile([P, 416], F32, tag="pc", name="trps")

    def ptr_emit(q):
        nc.tensor.matmul(trps_t[0][:, q * 40:q * 40 + 40],
                         lhsT=lgb[:, q * P:(q + 1) * P],
                         rhs=identity[0:40, 0:40],
                         is_transpose=True, start=True, stop=True)

    # transposed u staged to SBUF (DVE may read only ONE operand from PSUM)
    uT_sb = rwork.tile([P, 256], F32, tag="uTsb")

    def put_emit(q):
        nc.tensor.matmul(trps_t[0][:, 160 + q * 64:160 + q * 64 + 64],
                         lhsT=ub2[:, q * P:(q + 1) * P],
                         rhs=identity[0:64, 0:64],
                         is_transpose=True, start=True, stop=True)

    def ut_stage(h):
        # [128,128] PSUM->SBUF copy of two uT chunks on the scalar engine
        nc.scalar.activation(uT_sb[:, h * P:(h + 1) * P],
                             trps_t[0][:, 160 + h * P:160 + (h + 1) * P],
                             Identity)

    def argmax_emit(th, q):
        # masked u directly: mrep = (logit == rowmax) * uT, one fused op
        chunk = trps_t[0][:, q * 40 + 32 * th:q * 40 + 32 * th + 8]
        ut = uT_sb[:, q * 64 + 32 * th:q * 64 + 32 * th + 32]
        max8 = rwork.tile([P, E], F32, tag="mx8", bufs=8, name=f"mx8_{th}_{q}")
        nc.vector.max(out=max8, in_=chunk)
        nc.vector.scalar_tensor_tensor(
            mrep[th][:, q * ER:(q + 1) * ER].rearrange("p (e r) -> p e r", e=E),
            chunk[:, :, None].broadcast_to([P, E, R]),
            max8[:, 0:1],
            ut.rearrange("p (e r) -> p e r", e=E),
            IsEq, Mult)

    pm_ps = {}

    def pm_emit(th):
        # one [128,128] transpose flips all 4 masked-u chunks of a th;
        # chunk q's rows land at partition base 32q (all legal)
        pm_ps[th] = ppc.tile([P, P], BF16, tag="pc", name=f"pmb{th}")
        nc.tensor.matmul(pm_ps[th], lhsT=mrep[th], rhs=identity16,
                         is_transpose=True, start=True, stop=True)

    def um_emit(th):
        # stage the transposed masked-u into SBUF: th0 on scalar, th1 on
        # vector, so the two staging chains run concurrently
        if th == 0:
            for q in range(4):
                nc.scalar.activation(
                    um_stack[0:32, q * P:(q + 1) * P],
                    pm_ps[0][32 * q:32 * (q + 1), :], Identity)
            nc.scalar.activation(um_stack[64:96, :], um_stack[0:32, :],
                                 Identity)
        else:
            for q in range(4):
                nc.vector.tensor_copy(
                    um_stack[32:64, q * P:(q + 1) * P],
                    pm_ps[1][32 * q:32 * (q + 1), :])
            nc.vector.tensor_copy(um_stack[96:128, :], um_stack[32:64, :])

    def phase2(th, corr):
        for kk in range(KD):
            nc.tensor.matmul(corr, lhsT=cg8_sb[kk],
                             rhs=dx8t[th][:, kk * 512:(kk + 1) * 512],
                             start=(kk == 0), stop=(kk == KD - 1))

    # --- f1 mains with the dx correction + logit transposes + argmaxes
    # interleaved (dx8 is token-split so th1 never waits on the k4-7 DMA)
    ps_f[1] = f_tiles(1)
    for k in range(KD):
        f_mains(1, ps_f[1], [k])
        if k == 0:
            phase2(0, corr0)
        elif k == 1:
            # exact logits = x16-logits + 2^-18 * (fp8 dx @ fp8 g)
            nc.vector.scalar_tensor_tensor(lgb[0:8, :], corr0, CORR_DESCALE,
                                           lgb1[0:8, :], Mult, Add)
            phase2(1, corr1)
        elif k == 2:
            nc.vector.scalar_tensor_tensor(lgb[32:40, :], corr1, CORR_DESCALE,
                                           lgb1[32:40, :], Mult, Add)
        elif k == 3:
            trps_bank()
            ptr_emit(0)
            put_emit(0)
            ptr_emit(1)
            put_emit(1)
            ut_stage(0)
            argmax_emit(0, 0)
            argmax_emit(0, 1)
            argmax_emit(1, 0)
            argmax_emit(1, 1)
        elif k == 4:
            ptr_emit(2)
            put_emit(2)
            ptr_emit(3)
            put_emit(3)
            ut_stage(1)
            argmax_emit(0, 2)
            argmax_emit(0, 3)
            argmax_emit(1, 2)
            argmax_emit(1, 3)
        elif k == 6:
            pm_emit(0)
        elif k == 7:
            um_emit(0)

    # --- f2 mains with the th1 mask transpose + um interleaved
    ps_f[2] = f_tiles(2)
    for k in range(KD):
        f_mains(2, ps_f[2], [k])
        if k == 0:
            pm_emit(1)
        elif k == 1:
            um_emit(1)

    # --- close f0..f2, then steady-state f3..f31
    quad_stops(0, 1, [ps_f[0][0], ps_f[0][1], ps_f[1][0], ps_f[1][1]])
    f_acts(0, ps_f[0])
    f_acts(1, ps_f[1])
    prev = 2
    for f in range(3, KF):
        ps_f[f] = f_tiles(f)
        f_mains(f, ps_f[f], range(KD))
        if f % 2 == 1:
            quad_stops(prev, f, [ps_f[prev][0], ps_f[prev][1],
                                 ps_f[f][0], ps_f[f][1]])
            f_acts(prev, ps_f[prev])
            f_acts(f, ps_f[f])
            del ps_f[prev], ps_f[f]
            prev = None
        else:
            prev = f

    # ---------- matmul 2: outT = wo @ inter + wo_b ----------
    for d in range(KD):
        wo_big = wop.tile([P, F], BF16, tag="wo", name=f"wo{d}")
        nc.sync.dma_start(out=wo_big, in_=io["woTt"][d])
        ps = [pp.tile([P, 512], F32, tag="pb", name=f"p2_{d}_{th}")
              for th in range(TH)]
        orow = io["outT"][d * P:(d + 1) * P, :]
        osb0 = outp.tile([P, 512], BF16, tag="osb0")
        osb1 = outp.tile([P, 512], BF16, tag="osb1")
        last = d == KD - 1
        ths = ([(0,), (1,)] if last else [(0, 1)])
        for grp in ths:
            for kf in range(KF):
                for th in grp:
                    nc.tensor.matmul(ps[th],
                                     lhsT=wo_big[:, kf * P:(kf + 1) * P],
                                     rhs=inter_sb[kf][:, TS[th]],
                                     start=(kf == 0), stop=(kf == KF - 1))
            if last and grp == (0,):
                # th0's act+DMA drain under th1's matmul stream
                nc.scalar.activation(osb0, ps[0], Identity,
                                     bias=wob_sb[:, d:d + 1])
                nc.sync.dma_start(out=orow[:, TS[0]], in_=osb0)
        if last:
            # final drain: scalar act (faster), output split across S+A queues
            nc.scalar.activation(osb1, ps[1], Identity, bias=wob_sb[:, d:d + 1])
            nc.sync.dma_start(out=orow[:, 512:768], in_=osb1[:, 0:256])
            nc.scalar.dma_start(out=orow[:, 768:1024], in_=osb1[:, 256:512])
        else:
            nc.vector.tensor_scalar(osb1, ps[1], wob_sb[:, d:d + 1], None, Add)
            nc.sync.dma_start(out=orow[:, TS[1]], in_=osb1)
            nc.scalar.activation(osb0, ps[0], Identity, bias=wob_sb[:, d:d + 1])
            nc.sync.dma_start(out=orow[:, TS[0]], in_=osb0)


_CACHED_NC = None


def build_nc():
    global _CACHED_NC
    if _CACHED_NC is not None:
        return _CACHED_NC
    nc = bacc.Bacc("TRN2", target_bir_lowering=False, debug=False,
                   enable_asserts=False, num_devices=NCORES)
    decls = [
        ("xT16", [D, N], BF16, False),
        ("dx8a", [P, KD * 512], F8E4, False),
        ("dx8b", [P, KD * 512], F8E4, False),
        ("cgTt", [P, KD * 72], BF16, False),
        ("cg8t", [P, KD * E], F8E4, False),
        ("biases", [P, 64], F32, False),
        ("bT4", [P, F], BF16, False),
        ("wif0t", [P, KD * P], BF16, False),
        ("wiTt", [8, P, KD * 512], BF16, False),
        ("woTt", [KD, P, F], BF16, False),
        ("outT", [D, N], BF16, True),
    ]
    io = {}
    for name, shape, dt_, is_out in decls:
        io[name] = nc.dram_tensor(
            name, shape, dt_, kind="ExternalOutput" if is_out else "ExternalInput"
        ).ap()
    with tile.TileContext(nc) as tc:
        with ExitStack() as ctx:
            _emit(ctx, tc, io)
    nc.compile()
    _CACHED_NC = nc
    return nc


def make_in_maps(inputs: dict) -> list[dict]:
    f32 = np.float32
    x = np.ascontiguousarray(np.asarray(inputs["hidden_states"], f32).reshape(NT, D))
    gT = np.asarray(inputs["gate_W"], f32).T                                # [D, E]
    aT = np.asarray(inputs["lora_A"], f32).reshape(ER, D).T                 # [D, 32]
    ga = np.concatenate([aT, gT], axis=1)                                   # [D, 40]
    ga16 = ga.astype(BF)
    dga16 = (ga - ga16.astype(f32)).astype(BF)
    cgT = np.concatenate(
        [ga16, np.zeros((D, 24), BF), dga16[:, 32:40]], axis=1)             # [D, 72]
    # pre-tiled straight-copy layouts: per-partition rows contiguous in HBM
    cgTt = np.ascontiguousarray(
        cgT.reshape(KD, P, 72).transpose(1, 0, 2).reshape(P, KD * 72))
    cg8T = np.clip(gT * G8_SCALE, -224.0, 224.0).astype(F8)                 # [D, 8]
    cg8t = np.ascontiguousarray(
        cg8T.reshape(KD, P, E).transpose(1, 0, 2).reshape(P, KD * E))
    biases = np.zeros((P, 64), f32)
    biases[:, 0:KF] = np.asarray(inputs["wi_b"], f32).reshape(KF, P).T
    biases[:, KF:KF + KD] = np.asarray(inputs["wo_b"], f32).reshape(KD, P).T
    biases[0:E, 40] = np.asarray(inputs["gate_b"], f32)
    bT = np.asarray(inputs["lora_B"], f32).transpose(0, 2, 1).reshape(ER, F)
    bT4 = np.ascontiguousarray(np.tile(bT.astype(BF), (4, 1)))              # [128, F]
    wiT3 = np.asarray(inputs["wi_W"], f32).T.astype(BF).reshape(KD, P, F)
    wif0t = np.ascontiguousarray(
        wiT3[:, :, 0:P].transpose(1, 0, 2).reshape(P, KD * P))
    wiTt = np.ascontiguousarray(
        wiT3.reshape(KD, P, 8, 512).transpose(2, 1, 0, 3).reshape(8, P, KD * 512))
    woT = np.asarray(inputs["wo_W"], f32).T.astype(BF)                      # [F, D]
    # pre-tiled to SBUF layout: woTt[d, p, kf*128+j] = woT[kf*128+p, d*128+j]
    woTt = np.ascontiguousarray(
        woT.reshape(KF, P, KD, P).transpose(2, 1, 0, 3).reshape(KD, P, F))

    in_maps = []
    for c in range(NCORES):
        xT32 = np.ascontiguousarray(x[c * N:(c + 1) * N].T)                 # [D, N]
        xT16 = xT32.astype(BF)
        dxT8 = np.clip((xT32 - xT16.astype(f32)) * DX_SCALE,
                       -224.0, 224.0).astype(F8)                            # [D, N]
        dx83 = dxT8.reshape(KD, P, N)
        dx8a = np.ascontiguousarray(
            dx83[:, :, 0:512].transpose(1, 0, 2).reshape(P, KD * 512))
        dx8b = np.ascontiguousarray(
            dx83[:, :, 512:1024].transpose(1, 0, 2).reshape(P, KD * 512))
        in_maps.append({
            "xT16": np.ascontiguousarray(xT16),
            "dx8a": dx8a, "dx8b": dx8b,
            "cgTt": cgTt, "cg8t": cg8t, "biases": biases, "bT4": bT4,
            "wif0t": wif0t, "wiTt": wiTt, "woTt": woTt,
        })
    return in_maps


def kernel(**inputs) -> np.ndarray:
    nc = build_nc()
    in_maps = make_in_maps(inputs)
    res = run_bass_kernel_spmd(nc, in_maps, core_ids=list(range(NCORES)))
    out = np.empty((NT, D), np.float32)
    for c in range(NCORES):
        out[c * N:(c + 1) * N] = res.results[c]["outT"].T.astype(np.float32)
    return out.reshape(B, S, D)
